# revision 68
# baseline (speedup 1.0000x reference)
"""ChunkCrossAttention Trainium2 kernel.

Math (per reference):
  x = chunk_embeddings[0]                      # (S, L)
  k, v = split(x @ W_kv.T)                     # (S, D) each
  scores = einsum('jqd,sd->jqs', q, k) / sqrt(D), masked
  attn = softmax(scores, -1)
  out = (attn @ v) @ W_out.T + q  -> LayerNorm(gamma, beta)

Strategy (8 NeuronCores) — AllGather-KV:
  - KV projection sharded over S: each core projects its own 512 keys
    (k^T, v^T in [d, s] layout straight out of the PE).
  - W_out folded into v: v' = v @ W_out.T, with two ones columns
    appended so the attention matmul also emits the softmax denominator.
  - Each core writes its K^T / v' block (526 KB bf16) to DRAM and ONE
    AllGather (Shared pair-HBM output = the fast collective path)
    replicates all 4096 keys everywhere.  This is the only collective:
    partial-sum exchange, cross-core reduction and their per-collective
    latency floors are gone entirely.
  - Each core then attends its own 1024 query rows over all 4096 keys
    (same FLOPs as the key-sharded variant), so softmax normalization,
    residual and LayerNorm are purely local.  Gathered K/V blocks are
    DMA'd to SBUF per key-block, so attention starts as soon as the
    first block lands.
  - Softmax runs without max-subtraction (scores ~ N(0,1), exp is safe
    in f32), mask folded into the Exp bias.
  - Attention inner loop is software-pipelined (sc_i then av_{i-1}) so
    the PE never waits on the scalar engine's Exp.
"""
import sys

sys.path.insert(0, "/opt/trn_rl_repo")

import numpy as np

import concourse.bacc as bacc
import concourse.mybir as mybir
import concourse.tile as tile
from concourse.bass_utils import run_bass_kernel_spmd

N_CORES = 8
J, Q, D = 64, 128, 256
S, L = 4096, 4096
S_LOC = S // N_CORES          # 512 keys per core
QALL = J * Q                  # 8192 query rows total
QR = QALL // N_CORES          # 1024 query rows per core (output shard)
DP = D + 2                    # attention free dim: D outputs + denom + pad
KELEM = 2 * 128 * 512         # K^T elems in the kv blob
VELEM = 4 * 128 * DP          # v' elems in the kv blob
LN_EPS = 1e-5
SCALE = 1.0 / np.sqrt(D)

F32 = mybir.dt.float32
BF16 = mybir.dt.bfloat16
AF = mybir.ActivationFunctionType
ALU = mybir.AluOpType


def build_program():
    nc = bacc.Bacc(None, num_devices=N_CORES)

    # inputs are partition-major so every DMA descriptor moves a multi-KB
    # contiguous stripe per partition (1KB row-major packets cap the fabric
    # at ~250 GB/s): element [p, a, s] = row p*32+a of the logical matrix —
    # any row permutation is fine for the contraction as long as x and w
    # share it.
    xT = nc.declare_dram_parameter("xT", [128, L // 128, S_LOC], BF16,
                                   isOutput=False)
    # wkvT split into K-half / V-half blocks so the K projection (and its
    # AllGather doorbell) completes ~22us before the V-half finishes
    wkvT = nc.declare_dram_parameter("wkvT", [2, 128, L // 128, D], BF16,
                                     isOutput=False)
    qT = nc.declare_dram_parameter("qT", [128, 2, QR], BF16, isOutput=False)
    ident = nc.declare_dram_parameter("ident", [128, 128], BF16,
                                      isOutput=False)
    woutT = nc.declare_dram_parameter("woutT", [D, D], BF16, isOutput=False)
    maskb = nc.declare_dram_parameter("maskb", [128, S // 128], F32,
                                      isOutput=False)
    gamma = nc.declare_dram_parameter("gamma", [D], F32, isOutput=False)
    beta = nc.declare_dram_parameter("beta", [D], F32, isOutput=False)
    y = nc.declare_dram_parameter("y", [QR, D], F32, isOutput=True)

    # flat 1-D collective buffers: the AllGather's latency degrades badly
    # when handed a multi-dim access pattern; layout is partition-major
    # [(p dc s)] / [(p ss f)] so publish/read DMAs still move 2KB stripes
    k_loc = nc.dram_tensor("k_loc", [KELEM], BF16)
    k_sh = nc.dram_tensor("k_sh", [N_CORES, KELEM], BF16,
                          addr_space="Shared")
    v_loc = nc.dram_tensor("v_loc", [VELEM], BF16)
    v_sh = nc.dram_tensor("v_sh", [N_CORES, VELEM], BF16,
                          addr_space="Shared")
    # tiny warmup AllGathers: the first rings at ~13us so the collective
    # stream's one-time setup cost burns during phase 1; the second rings
    # near phase-1's end so the stream is awake when the K doorbell lands
    warm_loc = nc.dram_tensor("warm_loc", [128], BF16)
    warm_sh = nc.dram_tensor("warm_sh", [N_CORES, 128], BF16,
                             addr_space="Shared")
    warm2_loc = nc.dram_tensor("warm2_loc", [128], BF16)
    warm2_sh = nc.dram_tensor("warm2_sh", [N_CORES, 128], BF16,
                              addr_space="Shared")

    import concourse.bass as bass

    with tile.TileContext(nc) as tc:
        with tc.tile_pool(name="singles", bufs=1) as singles, \
             tc.tile_pool(name="kv", bufs=1) as kvp, \
             tc.tile_pool(name="exp0", bufs=33) as epool0, \
             tc.tile_pool(name="hpool", bufs=2) as hpool, \
             tc.tile_pool(name="small", bufs=8) as small:

            # ---- constant tiles (loads pinned into mid-phase-1 below:
            # none are needed before ~t=55us, and at the head of the queue
            # they congest the DMA fabric during stream startup) ----
            woutT_sb = singles.tile([128, 2, D], BF16)
            maskb_sb = singles.tile([128, S // 128], F32)
            gamma_sb = singles.tile([128, D], F32)
            beta_sb = singles.tile([128, D], F32)
            ident_sb = singles.tile([128, 128], BF16)
            eps_sb = singles.tile([128, 1], F32)
            nc.vector.memset(eps_sb, LN_EPS)
            warm_sb = small.tile([128, 128], BF16, tag="warm")
            nc.vector.memset(warm_sb, 0.0)
            nc.gpsimd.dma_start(out=warm_loc[:], in_=warm_sb[0:1, :])
            nc.gpsimd.collective_compute(
                "AllGather", ALU.bypass,
                replica_groups=[list(range(N_CORES))],
                ins=[warm_loc[:]], outs=[warm_sh[:, :]], unique_tensors="Yes")
            qT_sb = singles.tile([128, 2, QR], BF16)
            qres_sb = singles.tile([128, QR // 128, D], F32)

            # ---- phase 1: local K^T / V^T projection over the S shard ----
            # x on the sync queue, w on the scalar queue; first chunk small
            # so the PE starts as early as possible.
            ps1 = tc.tile_pool(name="ps_kv", bufs=1, space="PSUM")
            ps_kv = ps1.__enter__()
            acc = [ps_kv.tile([128, S_LOC], F32, tag=f"acc{h}", name=f"acc{h}")
                   for h in range(4)]
            NA = L // 128
            xt_all = singles.tile([128, NA, S_LOC], BF16)
            wtk = singles.tile([128, NA, D], BF16)
            wtv = singles.tile([128, NA, D], BF16)
            chunks = [(0, 1), (1, 1), (2, 2), (4, 4), (8, 4), (12, 4),
                      (16, 8), (24, 8)]
            for ci, (a0, na) in enumerate(chunks):
                xt = xt_all[:, a0:a0 + na, :]
                nc.sync.dma_start(out=xt, in_=xT[:, a0:a0 + na, :])
                nc.scalar.dma_start(out=wtk[:, a0:a0 + na, :],
                                    in_=wkvT[0, :, a0:a0 + na, :])
                for a in range(a0, a0 + na):
                    for h in range(2):
                        nc.tensor.matmul(
                            acc[h], wtk[:, a, h * 128:(h + 1) * 128],
                            xt_all[:, a, :], start=(a == 0),
                            stop=(a == NA - 1))
                if a0 == 8:
                    # V-half weights pinned here: late enough to keep the
                    # startup fabric clear, early enough to finish the V
                    # projection just after K publishes
                    wpr = small.tile([128, 1], BF16, tag="wpr")
                    nc.vector.tensor_add(out=wpr, in0=wtv[:, 0, 0:1],
                                         in1=xt[:, 0, 0:1])
                    nc.scalar.dma_start(out=wtv[:, 0:16, :],
                                        in_=wkvT[1, :, 0:16, :])
                    nc.scalar.dma_start(out=wtv[:, 16:32, :],
                                        in_=wkvT[1, :, 16:32, :])
                if a0 == 4:
                    # probe chain reads every target (WAR) plus this x
                    # chunk (RAW) so all five constant loads fire here
                    pr = small.tile([128, 1], BF16, tag="pr")
                    nc.vector.tensor_add(out=pr, in0=woutT_sb[:, 0, 0:1],
                                         in1=xt[:, 0, 0:1])
                    prf = small.tile([128, 1], F32, tag="prf")
                    nc.vector.tensor_add(out=prf, in0=maskb_sb[:, 0:1],
                                         in1=pr)
                    nc.vector.tensor_add(out=prf, in0=gamma_sb[:, 0:1],
                                         in1=prf)
                    nc.vector.tensor_add(out=prf, in0=beta_sb[:, 0:1],
                                         in1=prf)
                    nc.vector.tensor_add(out=pr, in0=ident_sb[:, 0:1],
                                         in1=prf)
                    nc.gpsimd.dma_start(
                        out=woutT_sb,
                        in_=woutT.rearrange("(dc p) d2 -> p dc d2", p=128))
                    nc.gpsimd.dma_start(out=maskb_sb, in_=maskb[:, :])
                    g_ap = gamma[:]
                    nc.gpsimd.dma_start(out=gamma_sb, in_=bass.AP(
                        tensor=g_ap.tensor, offset=g_ap.offset,
                        ap=[[0, 128], g_ap.ap[0]]))
                    b_ap = beta[:]
                    nc.gpsimd.dma_start(out=beta_sb, in_=bass.AP(
                        tensor=b_ap.tensor, offset=b_ap.offset,
                        ap=[[0, 128], b_ap.ap[0]]))
                    nc.gpsimd.dma_start(out=ident_sb, in_=ident[:, :])
                if a0 == 16:
                    # qT pinned mid-stream: it costs phase 1 a few us of
                    # bandwidth, but loading it during the AllGather would
                    # contend with the collective's own DMA steps
                    # (measured much worse)
                    probe = small.tile([128, 1], BF16, tag="probe")
                    nc.vector.tensor_add(out=probe, in0=qT_sb[:, 0, 0:1],
                                         in1=xt[:, 0, 0:1])
                    nc.gpsimd.dma_start(out=qT_sb, in_=qT[:, :, :])


            kT_loc = kvp.tile([128, 2, S_LOC], BF16)
            nc.scalar.copy(out=kT_loc[:, 0, :], in_=acc[0])
            nc.scalar.copy(out=kT_loc[:, 1, :], in_=acc[1])
            # publish + AllGather K immediately — scores only need K, so
            # this collective runs while v' is still being folded
            nc.sync.dma_start(
                out=k_loc[:].rearrange("(p dc s) -> p dc s", p=128, dc=2),
                in_=kT_loc)
            nc.gpsimd.collective_compute(
                "AllGather", ALU.bypass,
                replica_groups=[list(range(N_CORES))],
                ins=[k_loc[:]], outs=[k_sh[:, :]], unique_tensors="Yes")



            # V-half projection: runs on the PE while the K AllGather is in
            # flight (its weights streamed during the K phase)
            for a in range(NA):
                for h in range(2):
                    nc.tensor.matmul(
                        acc[2 + h], wtv[:, a, h * 128:(h + 1) * 128],
                        xt_all[:, a, :], start=(a == 0), stop=(a == NA - 1))

            vT_loc = kvp.tile([128, 2, S_LOC], BF16)
            nc.vector.tensor_copy(out=vT_loc[:, 0, :], in_=acc[2])
            nc.vector.tensor_copy(out=vT_loc[:, 1, :], in_=acc[3])

            # ---- v' = v @ W_out.T, plus ones columns -> [s, DP] ----
            vp_sb = kvp.tile([128, 4, DP], BF16)
            nc.vector.memset(vp_sb, 1.0)
            for ss in range(4):
                pv = ps_kv.tile([128, D], F32, tag=f"pv{ss % 2}",
                                name=f"pv{ss % 2}")
                for dc in range(2):
                    nc.tensor.matmul(
                        pv, vT_loc[:, dc, ss * 128:(ss + 1) * 128],
                        woutT_sb[:, dc, :], start=(dc == 0), stop=(dc == 1))
                nc.vector.tensor_copy(out=vp_sb[:, ss, 0:D], in_=pv)

            # residual rows derived on-chip (PE transpose of qT in the
            # AllGather latency hole) instead of a separate 1MB qres load
            for t in range(QR // 128):
                for dc in range(2):
                    tr = ps_kv.tile([128, 128], BF16, tag=f"tr{dc}",
                                    name=f"tr{dc}")
                    nc.tensor.transpose(tr, qT_sb[:, dc, t * 128:(t + 1) * 128],
                                        ident_sb)
                    nc.vector.tensor_copy(
                        out=qres_sb[:, t, dc * 128:(dc + 1) * 128], in_=tr)
            ps1.__exit__(None, None, None)

            nc.sync.dma_start(
                out=v_loc[:].rearrange("(p ss f) -> p ss f", p=128, ss=4),
                in_=vp_sb)
            nc.gpsimd.collective_compute(
                "AllGather", ALU.bypass,
                replica_groups=[list(range(N_CORES))],
                ins=[v_loc[:]], outs=[v_sh[:, :]], unique_tensors="Yes")

            # gathered K/v' -> SBUF, one DMA pair per key-block so the
            # attention pipeline starts on block 0 immediately
            kT_all = kvp.tile([128, N_CORES, 2, 512], BF16)
            vp_all = kvp.tile([128, N_CORES, 4, DP], BF16)
            nc.sync.dma_start(
                out=kT_all,
                in_=k_sh.rearrange("r (p dc s) -> p r dc s", p=128, dc=2))
            nc.gpsimd.dma_start(
                out=vp_all,
                in_=v_sh.rearrange("r (p ss f) -> p r ss f", p=128, ss=4))

            # ---- phase 2: attention for our 1024 rows over all keys ----
            ps3 = tc.tile_pool(name="ps_at", bufs=1, space="PSUM")
            ps_at = ps3.__enter__()
            ps3b = tc.tile_pool(name="ps_sc", bufs=4, space="PSUM")
            ps_sc = ps3b.__enter__()

            NST = S // 128                            # 32 key tiles

            def scores(row0, i):
                blk, st = i // 4, i % 4
                sc = ps_sc.tile([128, 512], F32, tag="sc")
                for dc in range(2):
                    nc.tensor.matmul(
                        sc, kT_all[:, blk, dc, st * 128:(st + 1) * 128],
                        qT_sb[:, dc, row0:row0 + 512],
                        start=(dc == 0), stop=(dc == 1))
                ex = epool0.tile([128, 512], BF16, tag="ex")
                nc.scalar.activation(out=ex, in_=sc, func=AF.Exp,
                                     bias=maskb_sb[:, i:i + 1], scale=SCALE)
                return ex

            def av(at, ex, i):
                blk, st = i // 4, i % 4
                for qt in range(4):
                    nc.tensor.matmul(
                        at[qt], ex[i][:, qt * 128:(qt + 1) * 128],
                        vp_all[:, blk, st, :],
                        start=(i == 0), stop=(i == NST - 1))

            def epilogue(at, row0):
                h_half = hpool.tile([128, 4, D], F32, tag="h")
                for qt in range(4):
                    hs = h_half[:, qt, :]
                    rec = small.tile([128, 1], F32, tag="rec")
                    nc.vector.reciprocal(out=rec, in_=at[qt][:, D:D + 1])
                    nc.vector.scalar_tensor_tensor(
                        out=hs, in0=at[qt][:, 0:D], scalar=rec,
                        in1=qres_sb[:, row0 // 128 + qt, :],
                        op0=ALU.mult, op1=ALU.add)
                    stats = small.tile([128, 6], F32, tag="stats")
                    nc.vector.bn_stats(out=stats, in_=hs)
                    mv = small.tile([128, 2], F32, tag="mv")
                    nc.vector.bn_aggr(out=mv, in_=stats)
                    rstd = small.tile([128, 1], F32, tag="rstd")
                    nc.scalar.activation(out=rstd, in_=mv[:, 1:2], func=AF.Sqrt,
                                         bias=eps_sb, scale=1.0)
                    nc.vector.reciprocal(out=rstd, in_=rstd)
                    nc.vector.tensor_scalar(out=hs, in0=hs,
                                            scalar1=mv[:, 0:1], scalar2=rstd,
                                            op0=ALU.subtract, op1=ALU.mult)
                    # gamma/beta on gpsimd: takes ~3.4us of the exposed
                    # final-chunk epilogue off the vector engine
                    nc.gpsimd.tensor_mul(out=hs, in0=hs, in1=gamma_sb)
                    nc.gpsimd.tensor_add(out=hs, in0=hs, in1=beta_sb)
                nc.gpsimd.dma_start(
                    out=y[row0:row0 + 512, :].rearrange("(t p) d -> p t d",
                                                        p=128),
                    in_=h_half)

            # chunk-0 scores stream first (they only need K, and run while
            # the v' AllGather completes); chunk-1 scores interleave with
            # chunk-0 AVs so the PE never drains; chunk-0's epilogue hides
            # under chunk-1's AVs.
            at0 = [ps_at.tile([128, DP], F32, tag=f"at{i}", name=f"at{i}")
                   for i in range(4)]
            ex0 = [scores(0, i) for i in range(NST)]
            ex1 = [None] * NST
            for i in range(NST):
                ex1[i] = scores(512, i)
                av(at0, ex0, i)
            epilogue(at0, 0)
            at1 = [ps_at.tile([128, DP], F32, tag=f"at{i}", name=f"at{i}")
                   for i in range(4)]
            for i in range(NST):
                av(at1, ex1, i)
            epilogue(at1, 512)

            ps3b.__exit__(None, None, None)
            ps3.__exit__(None, None, None)

    nc.finalize()
    return nc


_NC_CACHE = None


def _make_in_maps(inputs):
    jq = np.asarray(inputs["justice_queries"], dtype=np.float32)
    x = np.asarray(inputs["chunk_embeddings"], dtype=np.float32)[0]
    mask = np.asarray(inputs["chunk_mask"])
    wkv = np.asarray(inputs["W_kv"], dtype=np.float32)
    wout = np.asarray(inputs["W_out"], dtype=np.float32)
    gamma = np.asarray(inputs["ln_gamma"], dtype=np.float32)
    beta = np.asarray(inputs["ln_beta"], dtype=np.float32)

    import ml_dtypes
    bf16 = ml_dtypes.bfloat16
    xT = np.ascontiguousarray(x.T.astype(bf16))         # (L, S)
    wkvT = np.ascontiguousarray(wkv.T.astype(bf16))     # (L, 2D)
    flat = np.ascontiguousarray(jq.reshape(J * Q, D))   # (8192, D)
    qT = np.ascontiguousarray(flat.T.astype(bf16))      # (D, 8192)
    woutT = np.ascontiguousarray(wout.T.astype(bf16))   # (D, D)
    mb_full = np.where(mask != 0, 0.0, -1e30).astype(np.float32)
    mb = np.ascontiguousarray(mb_full.reshape(S // 128, 128).T)

    # partition-major packing: [p, a, cols] with logical row = p*na + a for
    # x/w (any shared row permutation works for the contraction) and
    # row = a*128 + p for qT (must match the k/v layout d = dc*128+p)
    wkvT_p = np.ascontiguousarray(np.stack([
        wkvT[:, 0:D].reshape(128, L // 128, D),
        wkvT[:, D:2 * D].reshape(128, L // 128, D)]))

    in_maps = []
    for c in range(N_CORES):
        xc = xT[:, c * S_LOC:(c + 1) * S_LOC]
        qc = qT[:, c * QR:(c + 1) * QR]
        in_maps.append({
            "xT": np.ascontiguousarray(xc.reshape(128, L // 128, S_LOC)),
            "wkvT": wkvT_p,
            "qT": np.ascontiguousarray(
                qc.reshape(2, 128, QR).transpose(1, 0, 2)),
            "ident": np.eye(128, dtype=bf16),
            "woutT": woutT,
            "maskb": mb,
            "gamma": gamma,
            "beta": beta,
        })
    return in_maps


def kernel(**inputs) -> np.ndarray:
    global _NC_CACHE
    in_maps = _make_in_maps(inputs)
    if _NC_CACHE is None:
        _NC_CACHE = build_program()
    res = run_bass_kernel_spmd(_NC_CACHE, in_maps, list(range(N_CORES)))
    out = np.concatenate([res.results[c]["y"] for c in range(N_CORES)], axis=0)
    return np.ascontiguousarray(out.reshape(J, Q, D).astype(np.float32))
